# revision 15
# baseline (speedup 1.0000x reference)
"""Multi-head attention (B=4, L=2048, D=1024, H=16) on 8 Trainium2 NeuronCores.

Sharding: core c = (batch b = c//2, head-half hh = c%2). Each core computes
its 8 heads (4 pairs) for ALL 2048 queries of its batch against all 2048
keys. The q/k/v projections and the output projection are halved per core
(only this core's 512 head dims), and the output-projection reduction across
the two head-halves is done HOST-side during unsharding
(out[b] = part[2b] + part[2b+1] + bias) — no collectives (a collective in
the NEFF down-clocks the whole chip to 5/6 frequency, measured +21%).

All-bf16 data path (fp32 psum). Same software-pipelined S/exp/PV stream as
the query-split baseline: per item (ic, pair, key-chunk) the S pair
(tile_position row-packed, K=64) runs two items ahead of the exp and PV.
Blocks are ic-major so each ic-segment's normalization + output projection
streams into the next segment's slots. Value projection runs inside the
first block; k/q projection chunks stream deadline-driven from persistent
weight tiles; softmax normalization is batched per ic (vpa ones-column
yields Z in psum row 64, one reciprocal per ic). The output bias
(b_v @ w_o.T + b_o, exact since softmax rows sum to 1) is added host-side.
Query-split baseline (full-duplicate projections): ~453us.
"""

import sys

if "/opt/trn_rl_repo" not in sys.path:
    sys.path.insert(0, "/opt/trn_rl_repo")

import numpy as np

import concourse.bacc as bacc
import concourse.tile as tile
from concourse import mybir
from concourse.bass_utils import run_bass_kernel_spmd

N_CORES = 8
B, L, D = 4, 2048, 1024
NH, DH = 16, 64          # total heads, head dim
NHC = NH // 2            # heads per core
DHH = NHC * DH           # head dims per core (512)
LQ = L                   # query rows per core (all of the batch's queries)
F32 = mybir.dt.float32
F32R = mybir.dt.float32r
BF16 = mybir.dt.bfloat16

KC = D // 128            # 8 contraction chunks for q/k/v projections
KCO = DHH // 128         # 4 contraction chunks for the output projection
NJ = L // 128            # 16 key j-chunks
NI = LQ // 512           # 4 query i-chunks of 512
NPAIR = NHC // 2         # 4 head pairs per core
VST = 66                 # vpa per-head stride (64 cols + ones + pad)
EXPF = mybir.ActivationFunctionType.Exp


def build_program():
    nc = bacc.Bacc("TRN2", target_bir_lowering=False, debug=False,
                   num_devices=N_CORES)
    with tile.TileContext(nc) as tc:
        _emit(nc, tc)
    nc.compile()
    return nc


def _emit(nc, tc):
    from collections import deque
    from contextlib import ExitStack

    top = ExitStack()
    dram = top.enter_context(tc.tile_pool(name="dram", bufs=1, space="DRAM"))

    def din(shape, dt, name):
        return dram.tile(shape, dt, kind="ExternalInput", name=name,
                         uniquify=False)

    xqT = din([D, LQ], BF16, "xqT")
    xkT = din([D, L], BF16, "xkT")
    xvT = din([D, L], BF16, "xvT")
    wqT = din([D, DHH], BF16, "wqT")
    wkT = din([D, DHH], BF16, "wkT")
    wvT = din([D, DHH], BF16, "wvT")
    woT = din([DHH, D], BF16, "woT")
    bqc = din([128, KCO], F32, "bqc")
    bkc = din([128, KCO], F32, "bkc")
    c_sel = din([64, 64 * 2 * NI * NPAIR], F32R, "c_sel")
    out = dram.tile([LQ, D], F32, kind="ExternalOutput", name="out",
                    uniquify=False)

    # ---- persistent SBUF -------------------------------------------------
    pers = top.enter_context(tc.tile_pool(name="pers", bufs=1))
    kpT = [pers.tile([128, L], BF16, name=f"kpT{m}") for m in range(NPAIR)]
    qpT = [pers.tile([128, LQ], BF16, name=f"qpT{m}") for m in range(NPAIR)]
    vpa = [pers.tile([128, NHC * VST], BF16, name=f"vpa{m}")
           for m in range(NJ)]
    outU = [pers.tile([128, LQ], BF16, name=f"outU{m}") for m in range(NPAIR)]
    xk_sb = pers.tile([128, KC, L], BF16, name="xk_sb")
    xq_sb = pers.tile([128, KC, LQ], BF16, name="xq_sb")
    wk_sb = pers.tile([128, KC, DHH], BF16, name="wk_sb")
    wq_sb = pers.tile([128, KC, DHH], BF16, name="wq_sb")
    wv_sb = pers.tile([128, KC, DHH], BF16, name="wv_sb")
    wo_sb = pers.tile([128, KCO, D], BF16, name="wo_sb")
    sel_sb = pers.tile([64, 64 * 2 * NI * NPAIR], F32R, name="sel_sb")
    zall = pers.tile([64, 512], F32, name="zall")
    bq_sb = pers.tile([128, KCO], F32, name="bq_sb")
    bk_sb = pers.tile([128, KCO], F32, name="bk_sb")

    xkT_r = xkT.rearrange("(kc p) l -> p kc l", p=128)
    xqT_r = xqT.rearrange("(kc p) l -> p kc l", p=128)
    xvT_r = xvT.rearrange("(kc p) l -> p kc l", p=128)
    wqT_r = wqT.rearrange("(kc p) m -> p kc m", p=128)
    wkT_r = wkT.rearrange("(kc p) m -> p kc m", p=128)
    wvT_r = wvT.rearrange("(kc p) m -> p kc m", p=128)
    woT_r = woT.rearrange("(kc p) m -> p kc m", p=128)

    # ---- pools -----------------------------------------------------------
    # PSUM budget (8 banks): psA 2 x [128,1024] = 4, psO 2 tags x [65,512]
    # = 2, ppk 2 x [128,512] = 2.
    attn_ctx = ExitStack()
    pxv = attn_ctx.enter_context(tc.tile_pool(name="pxv", bufs=3))
    ppk = attn_ctx.enter_context(tc.tile_pool(name="ppk", bufs=2, space="PSUM"))
    psA = attn_ctx.enter_context(tc.tile_pool(name="psA", bufs=1, space="PSUM"))
    psO = attn_ctx.enter_context(tc.tile_pool(name="psO", bufs=1, space="PSUM"))
    pe = attn_ctx.enter_context(tc.tile_pool(name="pe", bufs=3))
    prc = attn_ctx.enter_context(tc.tile_pool(name="prc", bufs=2))
    fs = attn_ctx.enter_context(tc.tile_pool(name="fs", bufs=3))
    pn = attn_ctx.enter_context(tc.tile_pool(name="pn", bufs=1))
    rz = pn.tile([64, 512], F32R, name="rz")

    # ---- value projection ------------------------------------------------
    xv_pre = {}

    def stage_xv(m):
        xb = pxv.tile([128, KC, 128], BF16, tag="xv", name=f"xv{m}")
        nc.sync.dma_start(out=xb[:], in_=xvT_r[:, :, m * 128:(m + 1) * 128])
        xv_pre[m] = xb

    def emit_vp(m):
        # value projection for key chunk m -> vpa[m] (8 heads, 2 groups of 4)
        if m not in xv_pre:
            stage_xv(m)
        xb = xv_pre.pop(m)
        va = vpa[m].rearrange("p (h c) -> p h c", c=VST)
        nc.vector.memset(va[:, :, 64:66], 1.0)
        ps = ppk.tile([128, 512], F32, tag="pk", name=f"pv{m}")
        for kc in range(KC):
            nc.tensor.matmul(ps[:], xb[:, kc, :], wv_sb[:, kc, :],
                             start=(kc == 0), stop=(kc == KC - 1))
        nc.vector.tensor_copy(va[:, :, 0:64], ps[:])

    # ---- k/q projections (weights resident in SBUF) ----------------------
    def emit_kq_chunk(p, c):
        # c 0-3: kpT[p] key chunk c; c 4-7: qpT[p] query chunk c-4
        psl = slice(p * 128, (p + 1) * 128)
        if c < 4:
            w_sb, x_sb, dst, bias, cc = wk_sb, xk_sb, kpT[p], bk_sb, c
        else:
            w_sb, x_sb, dst, bias, cc = wq_sb, xq_sb, qpT[p], bq_sb, c - 4
        csl = slice(cc * 512, (cc + 1) * 512)
        ps = ppk.tile([128, 512], F32, tag="pk", name=f"pk{p}_{c}")
        for kc in range(KC):
            nc.tensor.matmul(ps[:], w_sb[:, kc, psl], x_sb[:, kc, csl],
                             start=(kc == 0), stop=(kc == KC - 1))
        nc.vector.tensor_scalar_add(dst[:, csl], ps[:], bias[:, p:p + 1])

    # ---- attention item stream (ic-major) --------------------------------
    items = [(ic, p, j) for ic in range(NI) for p in range(NPAIR)
             for j in range(NJ)]
    NIT = len(items)
    e_t = {}
    oz = {}
    # one 4-bank score tile; item k uses the (k%2) half. Scores for two
    # items are exp'd with a single ACTIVATE (saves the ~260ns
    # per-instruction ACT overhead — ACT is the binding engine in the
    # later segments).
    sAB = psA.tile([128, 2048], F32, name="sAB")

    def emit_S(k):
        ic, p, j = items[k]
        isl = slice(ic * 512, (ic + 1) * 512)
        jsl = slice(j * 128, (j + 1) * 128)
        base = (k % 2) * 1024
        nc.tensor.matmul(sAB[:, base:base + 512], kpT[p][0:64, jsl],
                         qpT[p][0:64, isl], tile_position=(0, 0))
        nc.tensor.matmul(sAB[:, base + 512:base + 1024], kpT[p][64:128, jsl],
                         qpT[p][64:128, isl], tile_position=(64, 0))

    def emit_exp_pair(k):
        # exp of items k and k+1 (k even) in one op
        e = pe.tile([128, 2048], BF16, tag="e")
        nc.scalar.activation(e[:], sAB[:], EXPF)
        e_t[k] = (e, 0)
        e_t[k + 1] = (e, 1024)

    def emit_PV(k):
        ic, p, j = items[k]
        hA, hB = 2 * p, 2 * p + 1
        if j == 0:
            ozA = psO.tile([65, 512], F32, tag="oa", name=f"ozA{p}_{ic}")
            ozB = psO.tile([65, 512], F32, tag="ob", name=f"ozB{p}_{ic}")
            oz[(p, ic)] = (ozA, ozB)
        ozA, ozB = oz[(p, ic)]
        et, eo = e_t.pop(k)
        e = et[:, eo:eo + 1024]
        if j == 0:
            # A first at the block start: the old ozA is released by the
            # (fast, ACT-side) outU copy before the old ozB is
            nc.tensor.matmul(ozA[:, :], vpa[j][:, hA * VST:hA * VST + 65],
                             e[:, 0:512], start=True, stop=False)
            nc.tensor.matmul(ozB[:, :], vpa[j][:, hB * VST:hB * VST + 65],
                             e[:, 512:1024], start=True, stop=False)
        else:
            nc.tensor.matmul(ozB[:, :], vpa[j][:, hB * VST:hB * VST + 65],
                             e[:, 512:1024], start=False, stop=(j == NJ - 1))
            nc.tensor.matmul(ozA[:, :], vpa[j][:, hA * VST:hA * VST + 65],
                             e[:, 0:512], start=False, stop=(j == NJ - 1))

    def emit_block_end(k):
        ic, p, j = items[k]
        ozA, ozB = oz.pop((p, ic))
        isl = slice(ic * 512, (ic + 1) * 512)
        # split the psO drain across ACT (idle here) and DVE so the next
        # block's start=True matmuls wait ~0.9us instead of ~1.6us
        zsa = prc.tile([65, 512], F32, tag="zs", name=f"zsa{p}_{ic}")
        zsb = prc.tile([65, 512], F32, tag="zs", name=f"zsb{p}_{ic}")
        nc.vector.tensor_copy(outU[p][0:64, isl], ozA[0:64, :])
        nc.vector.tensor_copy(zsa[64:65, :], ozA[64:65, :])
        nc.vector.tensor_copy(outU[p][64:128, isl], ozB[0:64, :])
        nc.vector.tensor_copy(zsb[64:65, :], ozB[64:65, :])
        r0 = 8 * ic + 2 * p
        nc.sync.dma_start(out=zall[r0:r0 + 1, :], in_=zsa[64:65, :])
        nc.sync.dma_start(out=zall[r0 + 1:r0 + 2, :], in_=zsb[64:65, :])

    # ---- per-ic normalization + output projection ------------------------
    def emit_recip():
        # full-table reciprocal; idempotent on rows not yet final
        with nc.allow_low_precision(reason="fp32r rounding of 1/Z"):
            nc.vector.reciprocal(rz[:], zall[:])

    def emit_norm(p, ic):
        r0 = 8 * ic + 2 * p
        isl = slice(ic * 512, (ic + 1) * 512)
        rzb = ppk.tile([128, 512], F32, tag="pk", name=f"rzb{p}_{ic}")
        nc.tensor.matmul(rzb[:, :], sel_sb[:, r0 * 64:(r0 + 2) * 64], rz[:])
        nc.vector.tensor_mul(outU[p][:, isl], outU[p][:, isl], rzb[:])

    def emit_ph3(n, m, tail=False, alt=False):
        # partial output projection for query chunk m, output half n. In
        # the tail, contract pair 3 last (its norm lands latest) and
        # alternate copy/DMA engines so the drain isn't single-queued.
        nsl = slice(n * 512, (n + 1) * 512)
        msl = slice(m * 128, (m + 1) * 128)
        ps = ppk.tile([128, 512], F32, tag="pk", name=f"pf{n}_{m}")
        order = (1, 2, 0, 3) if tail else (0, 1, 2, 3)
        for i, kc in enumerate(order):
            nc.tensor.matmul(ps[:], outU[kc][:, msl], wo_sb[:, kc, nsl],
                             start=(i == 0), stop=(i == KCO - 1))
        ost = fs.tile([128, 512], F32, tag="fs", name=f"fo{n}_{m}")
        if alt:
            nc.scalar.copy(ost[:], ps[:])
            nc.scalar.dma_start(out=out[msl, nsl], in_=ost[:])
        else:
            nc.vector.tensor_copy(ost[:], ps[:])
            nc.sync.dma_start(out=out[msl, nsl], in_=ost[:])

    # ---- DMA helpers for deadline-driven bulk loads ----------------------
    def dma_wk(p):
        psl = slice(p * 128, (p + 1) * 128)
        nc.sync.dma_start(out=wk_sb[:, :, psl], in_=wkT_r[:, :, psl])

    def dma_wq(p):
        psl = slice(p * 128, (p + 1) * 128)
        nc.sync.dma_start(out=wq_sb[:, :, psl], in_=wqT_r[:, :, psl])

    def dma_xk(c):
        csl = slice(c * 512, (c + 1) * 512)
        nc.sync.dma_start(out=xk_sb[:, :, csl], in_=xkT_r[:, :, csl])

    def dma_xq(c):
        csl = slice(c * 512, (c + 1) * 512)
        nc.sync.dma_start(out=xq_sb[:, :, csl], in_=xqT_r[:, :, csl])

    # ---- prefix (critical-path DMA order) --------------------------------
    dma_wk(0)
    dma_xk(0)
    dma_wq(0)
    dma_xq(0)
    nc.sync.dma_start(out=bq_sb[:], in_=bqc[:])
    nc.sync.dma_start(out=bk_sb[:], in_=bkc[:])
    emit_kq_chunk(0, 0)
    emit_kq_chunk(0, 4)
    dma_xk(1)
    nc.sync.dma_start(out=wv_sb[:], in_=wvT_r[:])
    for m_ in range(3):
        stage_xv(m_)

    # deadline-ordered work: kp[p] chunk c is consumed by S(16p + 4c)
    # emitted at slot 16p+4c-2; qp[p] chunk ic by S(64ic + 16p) emitted at
    # slot 64ic+16p-2. DMA units (kind != "kq") don't consume slot budget.
    kq_work = []
    for c in range(1, 4):
        kq_work.append((4 * c - 6, "kq", 0, c))
        if c >= 2:
            kq_work.append((4 * c - 12, "dxk", c, 0))
    for p in range(1, NPAIR):
        kq_work.append((16 * p - 12, "dwk", p, 0))
        kq_work.append((16 * p - 11, "dwq", p, 0))
        for c in range(4):
            kq_work.append((16 * p + 4 * c - 6, "kq", p, c))
        kq_work.append((16 * p - 6, "kq", p, 4))
    for ic in range(1, NI):
        kq_work.append((64 * ic - 14, "dxq", ic, 0))
        for p in range(NPAIR):
            kq_work.append((64 * ic + 16 * p - 6, "kq", p, 4 + ic))
    kq_work.append((40, "dwo", 0, 0))
    kq_work.append((44, "dsel", 0, 0))
    kq_work = deque(sorted(kq_work))

    defer_q = []
    defer_hold = [0]

    # ---- software-pipelined emission -------------------------------------
    emit_S(0)
    emit_S(1)
    emit_exp_pair(0)
    emit_S(2)
    emit_S(3)
    emit_exp_pair(2)
    nc.vector.memset(zall[:, :], 1.0)
    for k in range(NIT):
        ic_cur, p_cur, j_cur = items[k]
        # slack work first: at block starts this gives the previous block's
        # psO-drain copies a head start before our start=True matmuls
        took = 0
        while kq_work and took < 2 and (kq_work[0][0] <= k + 4
                                        or (k % 2 == 0 and took == 0)):
            _, kind_u, a_u, b_u = kq_work.popleft()
            if kind_u == "kq":
                emit_kq_chunk(a_u, b_u)
                took += 1
            elif kind_u == "dwk":
                dma_wk(a_u)
            elif kind_u == "dwq":
                dma_wq(a_u)
            elif kind_u == "dxk":
                dma_xk(a_u)
            elif kind_u == "dxq":
                dma_xq(a_u)
            elif kind_u == "dwo":
                nc.sync.dma_start(out=wo_sb[:], in_=woT_r[:])
            elif kind_u == "dsel":
                nc.sync.dma_start(out=sel_sb[:], in_=c_sel[:])
        if defer_q and k % 2 == 1 and k >= defer_hold[0]:
            fn = defer_q.pop(0)
            fn()
            if fn is emit_recip:
                # the reciprocal is a ~3.5us DVE op; hold the first norm
                # back so its PE-side selector matmul never waits on it
                defer_hold[0] = k + 5
        if ic_cur == 0 and p_cur == 0:
            emit_vp(j_cur)
            if j_cur + 3 < NJ:
                stage_xv(j_cur + 3)
        emit_PV(k)
        if k == NIT - 1 or items[k + 1][2] == 0:
            emit_block_end(k)
            if (ic_cur, p_cur) == (NI - 1, NPAIR - 2):
                # early-partial tail: pairs 0-2 of the last segment can
                # normalize while pair 3's block runs
                defer_q.append(emit_recip)
                for p_ in range(NPAIR - 1):
                    defer_q.append(lambda p=p_: emit_norm(p, NI - 1))
            elif p_cur == NPAIR - 1 and ic_cur < NI - 1:
                # segment ic_cur fully drained: normalize + project it,
                # streamed into the next segment's slots
                defer_q.append(emit_recip)
                for p_ in range(NPAIR):
                    defer_q.append(lambda p=p_, ic=ic_cur: emit_norm(p, ic))
                for n_ in range(2):
                    for m_ in range(4 * ic_cur, 4 * ic_cur + 4):
                        defer_q.append(lambda n=n_, m=m_: emit_ph3(n, m))
        if k % 2 == 1 and k + 4 < NIT:
            emit_S(k + 3)
            emit_S(k + 4)
            emit_exp_pair(k + 3)
    # tail: last pair's normalization, then the final output projection
    # with pair 3's contraction last so it overlaps the norm chain
    for fn in defer_q:
        fn()
    emit_recip()
    # first tail unit split: its pair-0/1/2 matmuls run under the recip +
    # last norm; only its final pair-3 matmul waits on them
    m0 = 4 * (NI - 1)
    ps0 = ppk.tile([128, 512], F32, tag="pk", name="pf_t0")
    for i, kc in enumerate((1, 2, 0)):
        nc.tensor.matmul(ps0[:], outU[kc][:, m0 * 128:(m0 + 1) * 128],
                         wo_sb[:, kc, 0:512], start=(i == 0), stop=False)
    emit_norm(NPAIR - 1, NI - 1)
    nc.tensor.matmul(ps0[:], outU[3][:, m0 * 128:(m0 + 1) * 128],
                     wo_sb[:, 3, 0:512], start=False, stop=True)
    ost0 = fs.tile([128, 512], F32, tag="fs", name="fo_t0")
    nc.vector.tensor_copy(ost0[:], ps0[:])
    nc.sync.dma_start(out=out[m0 * 128:(m0 + 1) * 128, 0:512], in_=ost0[:])
    for u in range(1, 8):
        n_, m_ = u % 2, 4 * (NI - 1) + u // 2
        emit_ph3(n_, m_, tail=True, alt=(u % 2 == 1))
    attn_ctx.close()
    top.close()


_NC_CACHE = None


def _get_program():
    global _NC_CACHE
    if _NC_CACHE is None:
        _NC_CACHE = build_program()
    return _NC_CACHE


def prep_in_maps(q, k, v, w_q, b_q, w_k, b_k, w_v, b_v, w_o, b_o):
    import ml_dtypes

    f = np.float32
    bf = ml_dtypes.bfloat16
    q, k, v = (np.asarray(t, f) for t in (q, k, v))
    w_q, w_k, w_v, w_o = (np.asarray(t, f) for t in (w_q, w_k, w_v, w_o))
    b_q, b_k = np.asarray(b_q, f), np.asarray(b_k, f)
    scale = 1.0 / np.sqrt(DH)
    # softmax weights sum to 1, so b_v contributes exactly b_v @ w_o.T to
    # the output; the whole bias is applied host-side after the reduction.
    bor = ((np.asarray(b_v, f) @ w_o.T) + np.asarray(b_o, f)).astype(f)
    c_sel = np.zeros((64, 64 * 2 * NI * NPAIR), f)
    for r in range(2 * NI * NPAIR):
        c_sel[r, r * 64:(r + 1) * 64] = 1.0
    per_b = {}
    for b in range(B):
        per_b[b] = (np.ascontiguousarray(q[b].T).astype(bf),
                    np.ascontiguousarray(k[b].T).astype(bf),
                    np.ascontiguousarray(v[b].T).astype(bf))
    per_h = {}
    for hh in range(2):
        hsl = slice(hh * DHH, (hh + 1) * DHH)
        per_h[hh] = {
            "wqT": np.ascontiguousarray((w_q[hsl, :] * scale).T).astype(bf),
            "wkT": np.ascontiguousarray(w_k[hsl, :].T).astype(bf),
            "wvT": np.ascontiguousarray(w_v[hsl, :].T).astype(bf),
            "woT": np.ascontiguousarray(w_o[:, hsl].T).astype(bf),
            "bqc": np.ascontiguousarray(
                (b_q[hsl] * scale).reshape(KCO, 128).T),
            "bkc": np.ascontiguousarray(b_k[hsl].reshape(KCO, 128).T),
        }
    in_maps = []
    for c in range(N_CORES):
        b, hh = c // 2, c % 2
        qTb, kTb, vTb = per_b[b]
        m = {"xqT": qTb, "xkT": kTb, "xvT": vTb, "c_sel": c_sel}
        m.update(per_h[hh])
        in_maps.append(m)
    return in_maps, bor


def run(in_maps, trace=False, **kw):
    nc = _get_program()
    return run_bass_kernel_spmd(nc, in_maps, list(range(N_CORES)),
                                trace=trace, **kw)


def gather_out(res, bor):
    out = np.empty((B, L, D), np.float32)
    for b in range(B):
        out[b] = res.results[2 * b]["out"]
        out[b] += res.results[2 * b + 1]["out"]
    out += bor[None, None, :]
    return out


def kernel(**inputs):
    in_maps, bor = prep_in_maps(**inputs)
    res = run(in_maps)
    return gather_out(res, bor)
